# revision 92
# baseline (speedup 1.0000x reference)
"""Trainium2 Bass kernel for nn_Net_5334349382149 (4-layer GATv2 GNN + config MLP).

V3 design (per-dst-slot edge layout):
  - dst-node sharding across 8 cores; per core, local nodes PERMUTED by
    in-degree and bucketed into 128-node groups; group g's edges laid out as
    slots [j][p] = (g*128+p)-th node's j-th in-edge (pad slots masked).
  - Scores via the identity att.leaky(h) = 0.6h' + 0.4*sgn_A|h'| (h' = att-
    folded h) and softmax cancellation of the dst-side linear term:
      w = exp(0.6*SL_src + 0.4*(sumA|h'| - sumB|h'|))
    SL = rowsum of gathered xl' (batched 3D reduce); gat_bias is a no-op
    through InstanceNorm and is dropped.
  - XR needs NO gather/expansion: dst == partition row, so xr broadcasts along
    the slot axis with a stride-0 AP.
  - Aggregation: per slot-tile matmul with lhsT = diag(w) (one broadcast DVE
    op per group), accumulated in PSUM; denominator = row-sum of w.
  - Only ONE dma_gather stream remains (xl rows by src), halving the dominant
    Q7 descriptor-generation cost vs the previous version.
"""
import os
import sys
import numpy as np

for p in ("/opt/trn_rl_repo", "/opt/pypackages"):
    if p not in sys.path and os.path.isdir(p):
        sys.path.append(p)

import concourse.bass as bass
import concourse.tile as tile
from concourse import bacc, mybir
from concourse.masks import make_identity
from concourse.bass_utils import run_bass_kernel_spmd

F32 = mybir.dt.float32
GDT = mybir.dt.bfloat16
I16 = mybir.dt.int16
I32 = mybir.dt.int32
AF = mybir.ActivationFunctionType
ALU = mybir.AluOpType
AX = mybir.AxisListType

NCORES = 8
HID = 256
EMB = 128
OPS = 120
OPF = 140
CF = 24


class Cfg:
    def __init__(self, NS, NP, TPDs, C, CP, nA):
        self.NS = NS            # real nodes per core
        self.NP = NP            # padded nodes per core (mult of 128)
        self.TPDs = tuple(TPDs)  # slot-tiles per dst group (shared by cores)
        self.C = C
        self.CP = CP
        self.nA = tuple(nA)
        self.NDT = NP // 128
        self.TT = sum(self.TPDs)
        self.L = self.TT * 128   # edge slots per core
        self.N = NS * NCORES
        self.NPG = NP * NCORES

    def key(self):
        return (self.NS, self.NP, self.TPDs, self.C, self.CP, self.nA)


# ---------------------------------------------------------------------------
# host preprocessing
# ---------------------------------------------------------------------------

def _wrap_idx(idx):
    n = len(idx)
    assert n % 16 == 0
    w = idx.astype(np.int16).reshape(n // 16, 16).T
    return np.tile(w, (8, 1))


def host_prep(d, cfg=None):
    import ml_dtypes
    bf16 = ml_dtypes.bfloat16
    f32 = np.float32
    N_IN = int(np.asarray(d['node_feat']).shape[0])
    C_IN = int(np.asarray(d['config_feat']).shape[0])

    # ---- parameter folding ----
    tbl = np.asarray(d['embed_table'], f32)
    nrm = np.sqrt((tbl * tbl).sum(-1, keepdims=True))
    tbl = tbl * np.minimum(1.0, 1.0 / (nrm + 1e-7))
    W1 = np.asarray(d['early_W1'], f32)
    T1 = (tbl @ W1[:EMB]).astype(f32)
    inv_std = (1.0 / (np.asarray(d['node_feat_std'], f32) + 1e-4)).astype(f32)
    w1b = (W1[EMB:] * inv_std[:, None]).astype(f32)
    b0 = (-(np.asarray(d['node_feat_mean'], f32) * inv_std) @ W1[EMB:]).astype(f32)

    perms, nAs = [], []
    rho = np.arange(HID)
    wl_l, wr_l, bl_l, br_l, ratt_l = [], [], [], [], []
    for i in range(4):
        att = np.asarray(d['gat_att'][i], f32)
        pos = np.where(att >= 0)[0]
        neg = np.where(att < 0)[0]
        perm = np.concatenate([pos, neg])
        nAs.append(len(pos))
        assert np.abs(att[perm]).min() > 1e-12
        wl_l.append((np.asarray(d['gat_Wl'][i], f32)[rho][:, perm]
                     * att[perm][None, :]).astype(f32))
        wr_l.append((np.asarray(d['gat_Wr'][i], f32)[rho][:, perm]
                     * att[perm][None, :]).astype(f32))
        bl_l.append(((np.asarray(d['gat_bl'][i], f32) * att)[perm]).astype(f32))
        br_l.append(((np.asarray(d['gat_br'][i], f32) * att)[perm]).astype(f32))
        ratt_l.append((1.0 / att[perm]).astype(f32))
        perms.append(perm)
        rho = perm

    cf_inv = (1.0 / (np.asarray(d['config_feat_std'], f32) + 1e-4)).astype(f32)
    LW1 = np.asarray(d['late_W1'], f32)
    w1c = (LW1[:CF] * cf_inv[:, None]).astype(f32)
    bc0 = (-(np.asarray(d['config_feat_mean'], f32) * cf_inv) @ LW1[:CF]).astype(f32)
    w1p = LW1[CF:][perms[3]].astype(f32)
    cfTh = np.zeros((CF, ((C_IN + 127) // 128) * 128), f32)
    cfTh[:, :C_IN] = np.asarray(d['config_feat'], f32).T

    # ---- edge sharding: per-dst-slot layout, degree-sorted node permutation ----
    NS = N_IN // NCORES
    NP_ = ((NS + 127) // 128) * 128
    NDT = NP_ // 128
    ei = np.asarray(d['edge_index']).astype(np.int64)
    src = np.concatenate([ei[0], np.arange(N_IN)])
    dst = np.concatenate([ei[1], np.arange(N_IN)])
    owner = dst // NS

    per_core = []
    for k in range(NCORES):
        m = owner == k
        sk = src[m]
        lk = (dst[m] - k * NS).astype(np.int64)
        deg = np.bincount(lk, minlength=NS)
        pi = np.argsort(-deg, kind='stable')
        inv_pi = np.empty(NS, np.int64)
        inv_pi[pi] = np.arange(NS)
        per_core.append(dict(sk=sk, lk=lk, deg=deg, pi=pi, inv_pi=inv_pi))

    # common (cross-core max) TPD per group
    TPDs = []
    for g in range(NDT):
        t = 1
        for k in range(NCORES):
            pc = per_core[k]
            dn = pc['deg'][pc['pi']]
            lo, hi = g * 128, min((g + 1) * 128, NS)
            if lo < NS:
                t = max(t, int(dn[lo:hi].max()))
        TPDs.append(t)

    cfg = Cfg(NS, NP_, TPDs, C_IN, ((C_IN + 127) // 128) * 128, nAs)
    TT, L = cfg.TT, cfg.L

    # permuted global padded row ids for sources
    inv_tables = [per_core[k]['inv_pi'] for k in range(NCORES)]

    per_core_out = []
    for k in range(NCORES):
        pc = per_core[k]
        # adjacency in new node order
        adj = [[] for _ in range(NP_)]
        for s_, l_ in zip(pc['sk'], pc['lk']):
            ks = s_ // NS
            gr = ks * NP_ + inv_tables[ks][s_ % NS]
            adj[pc['inv_pi'][l_]].append(gr)
        # pad slots point at the poisoned row (own core's last pad node):
        # its xl row is -1000 everywhere => w = exp(0.6*SL) = 0 exactly.
        assert NS < NP_
        pad_row = k * NP_ + (NP_ - 1)
        src_g = np.full(L, pad_row, np.int32)
        off = 0
        for g in range(NDT):
            tpd = cfg.TPDs[g]
            for p in range(128):
                n = g * 128 + p
                for j, s_ in enumerate(adj[n]):
                    src_g[(off + j) * 128 + p] = s_
            off += tpd
        per_core_out.append(dict(src_g=src_g))

    # ---- per-core input maps ----
    nf = np.asarray(d['node_feat'], f32)
    opc = np.asarray(d['node_opcode']).astype(np.int32)
    cfp = np.zeros((cfg.CP, CF), f32)
    cfp[:C_IN] = np.asarray(d['config_feat'], f32)

    # per-layer scalar columns: ratt (post-perm) packed [128, 2, 4] lo/hi
    ratt_pack = np.zeros((HID, 4), f32)
    for i in range(4):
        ratt_pack[:, i] = ratt_l[i]
    b0_pack = b0.reshape(HID, 1)

    bc0_pack = bc0.reshape(HID, 1)

    shared = {
        't1a': T1[:, :128].astype(bf16), 't1b': T1[:, 128:].astype(bf16),
        'w1ba': w1b[:128].astype(bf16), 'w1bb': w1b[128:].astype(bf16),
        'w2a': np.asarray(d['early_W2'], f32)[:128].astype(bf16),
        'w2b': np.asarray(d['early_W2'], f32)[128:].astype(bf16),
        'w1c': w1c,
        'w1pa': w1p[:128], 'w1pb': w1p[128:],
        'w2la': np.asarray(d['late_W2'], f32)[:128],
        'w2lb': np.asarray(d['late_W2'], f32)[128:],
        'predw': np.asarray(d['pred_W'], f32),
        'predb': np.asarray(d['pred_b'], f32).reshape(1, 1),
        'cft': cfTh,
        'rattp_lo': ratt_pack[:128].copy(), 'rattp_hi': ratt_pack[128:].copy(),
        'b0_lo': b0_pack[:128].copy(), 'b0_hi': b0_pack[128:].copy(),
        'bc0_lo': bc0_pack[:128].copy(), 'bc0_hi': bc0_pack[128:].copy(),
    }
    for i in range(4):
        shared[f'wl{i}a'] = wl_l[i][:128].astype(bf16)
        shared[f'wl{i}b'] = wl_l[i][128:].astype(bf16)
        shared[f'wr{i}a'] = wr_l[i][:128].astype(bf16)
        shared[f'wr{i}b'] = wr_l[i][128:].astype(bf16)
        # bl'+br' folded into stored xl rows (constant-per-channel shift of the
        # aggregated output is killed by InstanceNorm; scores need the sum)
        shared[f'blr{i}'] = (bl_l[i] + br_l[i]).reshape(1, HID).astype(bf16)

    in_maps = []
    for k in range(NCORES):
        pc = per_core[k]
        e = per_core_out[k]
        nfk = np.zeros((cfg.NP, OPF), f32)
        nfk[:NS] = nf[k * NS:(k + 1) * NS][pc['pi']]
        ok = np.zeros(cfg.NP, np.int32)
        ok[:NS] = opc[k * NS:(k + 1) * NS][pc['pi']]
        ohop = np.zeros((OPS, cfg.NP), f32)
        ohop[ok, np.arange(cfg.NP)] = 1.0
        ohop[:, NS:] = 0.0
        m = dict(shared)
        m['nfa'] = nfk.T[:128].copy().astype(bf16)
        m['nfb'] = nfk.T[128:].copy().astype(bf16)
        m['ohop'] = ohop.astype(bf16)
        m['srcidx'] = _wrap_idx(e['src_g'])
        in_maps.append(m)
    return cfg, in_maps


# ---------------------------------------------------------------------------
# program builder
# ---------------------------------------------------------------------------

def build_program(cfg: Cfg):
    nc = bacc.Bacc("TRN2", target_bir_lowering=False, debug=False,
                   num_devices=NCORES, num_swdge_queues=4)
    NP_, NS, NDT, TT, L = cfg.NP, cfg.NS, cfg.NDT, cfg.TT, cfg.L
    NT = NDT
    TPDmax = max(cfg.TPDs)
    REPL = [list(range(NCORES))]

    def din(name, shape, dt=F32):
        return nc.dram_tensor(name, list(shape), dt, kind="ExternalInput")

    SLE = HID            # gathered row width
    # ---- external inputs ----
    nfa_d = din('nfa', (128, NP_), GDT)
    nfb_d = din('nfb', (OPF - 128, NP_), GDT)
    t1_d = [din('t1a', (OPS, 128), GDT), din('t1b', (OPS, 128), GDT)]
    ohop_d = din('ohop', (OPS, NP_), GDT)
    srcidx_d = din('srcidx', (128, L // 16), I16)
    w1ba_d = din('w1ba', (128, HID), GDT)
    w1bb_d = din('w1bb', (OPF - 128, HID), GDT)
    w2_d = [din('w2a', (128, HID), GDT), din('w2b', (HID - 128, HID), GDT)]
    wl_d = [[din(f'wl{i}a', (128, HID), GDT), din(f'wl{i}b', (128, HID), GDT)]
            for i in range(4)]
    wr_d = [[din(f'wr{i}a', (128, HID), GDT), din(f'wr{i}b', (128, HID), GDT)]
            for i in range(4)]
    w1c_d = din('w1c', (CF, HID))
    w1p_d = [din('w1pa', (128, HID)), din('w1pb', (128, HID))]
    w2l_d = [din('w2la', (128, 128)), din('w2lb', (128, 128))]
    predw_d = din('predw', (128, 1))
    predb_d = din('predb', (1, 1))
    cft_d = din('cft', (CF, cfg.CP))
    blr_d = [din(f'blr{i}', (1, HID), GDT) for i in range(4)]
    rattp_d = [din('rattp_lo', (128, 4)), din('rattp_hi', (128, 4))]
    b0_d = [din('b0_lo', (128, 1)), din('b0_hi', (128, 1))]
    bc0_d = [din('bc0_lo', (128, 1)), din('bc0_hi', (128, 1))]
    out_d = nc.dram_tensor('out', [1, cfg.CP], F32, kind="ExternalOutput")

    # ---- internal DRAM ----
    xl_own = nc.dram_tensor('xl_own', [NP_, SLE], GDT)
    xl_full = nc.dram_tensor('xl_full', [cfg.NPG, SLE], GDT, addr_space="Shared")
    ar_in = [nc.dram_tensor(f'ar_in{i}', [128, 4], F32) for i in range(6)]
    ar_out = [nc.dram_tensor(f'ar_out{i}', [128, 4], F32, addr_space="Shared")
              for i in range(6)]
    pool_in = nc.dram_tensor('pool_in', [128, 4], F32)
    pool_out = nc.dram_tensor('pool_out', [128 * NCORES, 4], F32,
                              addr_space="Shared")

    with tile.TileContext(nc) as tc, __import__('contextlib').ExitStack() as ctx:
        const = ctx.enter_context(tc.tile_pool(name="const", bufs=1))
        big = ctx.enter_context(tc.tile_pool(name="big", bufs=1))
        work = ctx.enter_context(tc.tile_pool(name="work", bufs=3))
        col = ctx.enter_context(tc.tile_pool(name="col", bufs=6))
        psum = ctx.enter_context(tc.tile_pool(name="psum", bufs=2, space="PSUM"))

        # ------ constants ------
        ident = const.tile([128, 128], F32, tag="ident", name="ident")
        make_identity(nc, ident[:])
        ident_bf = const.tile([128, 128], GDT, tag="identbf", name="identbf")
        nc.vector.tensor_copy(ident_bf[:], ident[:])
        ones_row = const.tile([1, 128], GDT, tag="onesrow", name="onesrow")
        nc.gpsimd.memset(ones_row[:], 1.0)
        zero_col = const.tile([128, 1], F32, tag="zeroc", name="zeroc")
        nc.gpsimd.memset(zero_col[:], 0.0)
        nc.const_aps.aps[(F32, 0.0)] = zero_col[:]
        eps_col = const.tile([128, 1], F32, tag="epsc", name="epsc")
        nc.gpsimd.memset(eps_col[:], 1e-5)
        eps16_col = const.tile([128, 1], F32, tag="eps16c", name="eps16c")
        nc.gpsimd.memset(eps16_col[:], 1e-16)

        def load_const(dram, tag):
            t = const.tile(list(dram.shape), dram.dtype, tag=tag)
            nc.sync.dma_start(out=t[:], in_=dram[:])
            return t

        srcidx = load_const(srcidx_d, 'srcidx')
        t1t = [load_const(t1_d[j], f't1{j}') for j in range(2)]
        # poison row: SL = sum_c(-1000) dominates => w = exp(...) == 0
        negrow = const.tile([1, SLE], GDT, tag="negrow", name="negrow")
        nc.gpsimd.memset(negrow[:], -1000.0)
        ones_bfc = const.tile([128, 1], GDT, tag="onesbfc", name="onesbfc")
        nc.gpsimd.memset(ones_bfc[:], 1.0)
        w1ba = load_const(w1ba_d, 'w1ba')
        w1bb = load_const(w1bb_d, 'w1bb')
        w2 = [load_const(w2_d[j], f'w2{j}') for j in range(2)]
        wl = [[load_const(wl_d[i][j], f'wl{i}{j}') for j in range(2)] for i in range(4)]
        wr = [[load_const(wr_d[i][j], f'wr{i}{j}') for j in range(2)] for i in range(4)]
        w1c = load_const(w1c_d, 'w1c')
        w1p = [load_const(w1p_d[j], f'w1p{j}') for j in range(2)]
        w2l = [load_const(w2l_d[j], f'w2l{j}') for j in range(2)]
        predw = load_const(predw_d, 'predw')
        predb = load_const(predb_d, 'predb')
        blrt = [load_const(blr_d[i], f'blr{i}') for i in range(4)]
        rattp = [load_const(rattp_d[j], f'rattp{j}') for j in range(2)]
        b0c = [load_const(b0_d[j], f'b0{j}') for j in range(2)]
        bc0c = [load_const(bc0_d[j], f'bc0{j}') for j in range(2)]

        # ------ persistent big tiles ------
        raw = [big.tile([128, NP_], GDT, tag=f"raw{m}", name=f"raw{m}") for m in range(2)]
        xt = [big.tile([128, NP_], GDT, tag=f"x{m}", name=f"x{m}") for m in range(2)]
        xr_sb = big.tile([128, NT * HID], GDT, tag="xr_sb", name="xr_sb")

        blocks = [(s, min(s + 512, NP_)) for s in range(0, NP_, 512)]

        def stats_tiles(tagp):
            return [work.tile([128, max(len(blocks), NDT)], F32, tag=f"{tagp}{m}",
                              name=f"{tagp}{m}") for m in range(2)]

        gq = [0]

        def gather_rows(out3, in_dram, idx_tile, i0_idx, total, elem):
            done = 0
            while done < total:
                n = min(1024, total - done)
                nc.gpsimd.dma_gather(
                    out_ap=out3[:, done // 128:(done + n) // 128, :],
                    in_ap=in_dram[:],
                    idxs_ap=idx_tile[:, (i0_idx + done) // 16:(i0_idx + done + n) // 16],
                    num_idxs=n, num_idxs_reg=n, elem_size=elem,
                    single_packet=False, queue_num=gq[0] % 4)
                gq[0] += 1
                done += n

        def evac_block(dst_tile, src_ap, c0, c1, mc, st1, st2, blk_i,
                       scalar1=0.0, scalar2=None, op0=ALU.add, op1=ALU.add):
            def one(a, b, accum):
                kw = {}
                if accum:
                    kw['accum_out'] = st1[mc][:, blk_i:blk_i + 1]
                s2 = 0.0 if scalar2 is None else scalar2
                nc.vector.tensor_scalar(dst_tile[:, a:b], src_ap[:, a - c0:b - c0],
                                        scalar1, s2, op0, op1, **kw)
                if accum and st2 is not None:
                    sq = work.tile([128, 512], F32, tag="sqscr", name="sqscr")
                    nc.scalar.activation(sq[:, :b - a], dst_tile[:, a:b], AF.Square,
                                         accum_out=st2[mc][:, blk_i:blk_i + 1])
            if c0 >= NS:
                one(c0, c1, False)
            elif c1 <= NS:
                one(c0, c1, True)
            else:
                one(c0, NS, True)
                one(NS, c1, False)

        def stats_and_norm(st1, st2, ar_i, ar_o, ntotal, nblk):
            art = work.tile([128, 4], F32, tag="art", name="art")
            for m in range(2):
                nc.vector.tensor_reduce(art[:, 2 * m:2 * m + 1], st1[m][:, :nblk],
                                        AX.X, ALU.add)
                nc.vector.tensor_reduce(art[:, 2 * m + 1:2 * m + 2], st2[m][:, :nblk],
                                        AX.X, ALU.add)
            nc.sync.dma_start(out=ar_i[:], in_=art[:])
            nc.gpsimd.collective_compute(
                "AllReduce", ALU.add, replica_groups=REPL,
                ins=[ar_i[:]], outs=[ar_o[:]])
            arr = work.tile([128, 4], F32, tag="arr", name="arr")
            nc.sync.dma_start(out=arr[:], in_=ar_o[:])
            rs_l, nmr_l = [], []
            for m in range(2):
                mu = col.tile([128, 1], F32, tag="mu", name="mu")
                nc.vector.tensor_scalar(mu[:], arr[:, 2 * m:2 * m + 1],
                                        1.0 / ntotal, None, ALU.mult)
                mu2 = col.tile([128, 1], F32, tag="mu2", name="mu2")
                nc.scalar.activation(mu2[:], mu[:], AF.Square)
                var = col.tile([128, 1], F32, tag="var", name="var")
                nc.vector.scalar_tensor_tensor(var[:], arr[:, 2 * m + 1:2 * m + 2],
                                               1.0 / ntotal, mu2[:],
                                               ALU.mult, ALU.subtract)
                sd = col.tile([128, 1], F32, tag="sd", name="sd")
                nc.scalar.activation(sd[:], var[:], AF.Sqrt, bias=eps_col[:])
                rs = col.tile([128, 1], F32, tag="rs", name="rs")
                nc.vector.reciprocal(rs[:], sd[:])
                nmr = col.tile([128, 1], F32, tag="nmr", name="nmr")
                nc.vector.tensor_scalar(nmr[:], mu[:], rs[:], -1.0, ALU.mult, ALU.mult)
                rs_l.append(rs)
                nmr_l.append(nmr)
            return rs_l, nmr_l

        def norm_gelu(src_tiles, dst_tiles, rs_l, nmr_l):
            for m in range(2):
                nc.scalar.activation(dst_tiles[m][:], src_tiles[m][:], AF.Gelu,
                                     bias=nmr_l[m][:], scale=rs_l[m][:])

        # =================== early stage ===================
        early = tc.alloc_tile_pool(name="early", bufs=1)
        nfTa = early.tile([128, NP_], GDT, tag="nfTa", name="nfTa")
        nfTb = early.tile([OPF - 128, NP_], GDT, tag="nfTb", name="nfTb")
        ohop = early.tile([OPS, NP_], GDT, tag="ohop", name="ohop")
        nc.sync.dma_start(out=ohop[:], in_=ohop_d[:])
        nc.sync.dma_start(out=nfTa[:], in_=nfa_d[:])
        nc.sync.dma_start(out=nfTb[:], in_=nfb_d[:])

        st1 = stats_tiles("e1s1")
        st2 = stats_tiles("e1s2")
        for mc in range(2):
            for bi, (s, e) in enumerate(blocks):
                w = e - s
                ps = psum.tile([128, 512], F32, tag="mm", name="mm")
                nc.tensor.matmul(ps[:, :w], lhsT=w1ba[:, mc * 128:(mc + 1) * 128],
                                 rhs=nfTa[:, s:e], start=True, stop=False)
                nc.tensor.matmul(ps[:, :w], lhsT=w1bb[:, mc * 128:(mc + 1) * 128],
                                 rhs=nfTb[:, s:e], start=False, stop=False)
                nc.tensor.matmul(ps[:, :w], lhsT=t1t[mc][:], rhs=ohop[:, s:e],
                                 start=False, stop=True)
                evac_block(raw[mc], ps[:, :w], s, e, mc, st1, st2, bi,
                           scalar1=b0c[mc][:])
        early.release()
        rs_l, nmr_l = stats_and_norm(st1, st2, ar_in[0], ar_out[0], cfg.N, len(blocks))
        norm_gelu(raw, xt, rs_l, nmr_l)

        st1 = stats_tiles("e2s1")
        st2 = stats_tiles("e2s2")
        for mc in range(2):
            for bi, (s, e) in enumerate(blocks):
                w = e - s
                ps = psum.tile([128, 512], F32, tag="mm", name="mm")
                for kc in range(2):
                    nc.tensor.matmul(ps[:, :w], lhsT=w2[kc][:, mc * 128:(mc + 1) * 128],
                                     rhs=xt[kc][:, s:e], start=(kc == 0),
                                     stop=(kc == 1))
                evac_block(raw[mc], ps[:, :w], s, e, mc, st1, st2, bi)
        rs_l, nmr_l = stats_and_norm(st1, st2, ar_in[1], ar_out[1], cfg.N, len(blocks))
        norm_gelu(raw, xt, rs_l, nmr_l)

        # =================== GAT layers ===================
        toff = []   # group offsets into slot-tile axis
        o = 0
        for g in range(NDT):
            toff.append(o)
            o += cfg.TPDs[g]

        edge = tc.alloc_tile_pool(name="edge", bufs=3)
        edgex = tc.alloc_tile_pool(name="edgex", bufs=5)
        for li in range(4):
            nA = cfg.nA[li]

            # ---- xl (node-major direct): lhsT = x_cm slices, rhs = W rows;
            #      output column 256 = SL (row-sum channel) ----
            for t in range(NT):
                ps = psum.tile([128, HID], F32, tag="nmm", name="nmm")
                nc.tensor.matmul(ps[:], lhsT=ones_row[:], rhs=blrt[li][:],
                                 start=True, stop=False)
                for kc in range(2):
                    nc.tensor.matmul(ps[:], lhsT=xt[kc][:, t * 128:(t + 1) * 128],
                                     rhs=wl[li][kc][:], start=False, stop=(kc == 1))
                xlt = work.tile([128, HID], GDT, tag="xlt", name="xlt")
                nc.scalar.activation(xlt[:], ps[:], AF.Copy)
                nc.sync.dma_start(out=xl_own[t * 128:(t + 1) * 128, :], in_=xlt[:])
            # poison the last pad row so pad-slot gathers yield w == 0
            nc.sync.dma_start(out=xl_own[NP_ - 1:NP_, :], in_=negrow[:])
            nc.gpsimd.collective_compute(
                "AllGather", ALU.bypass, replica_groups=REPL,
                ins=[xl_own[:]], outs=[xl_full[:]])

            # ---- xr (node-major direct, stays in SBUF) ----
            for t in range(NT):
                ps = psum.tile([128, HID], F32, tag="nmm", name="nmm")
                for kc in range(2):
                    nc.tensor.matmul(ps[:], lhsT=xt[kc][:, t * 128:(t + 1) * 128],
                                     rhs=wr[li][kc][:], start=(kc == 0), stop=(kc == 1))
                nc.scalar.activation(xr_sb[:, t * HID:(t + 1) * HID], ps[:], AF.Copy)

            # ---- edge phase ----
            st1 = stats_tiles("gs1")
            st2 = stats_tiles("gs2")
            for g in range(NDT):
                tpd = cfg.TPDs[g]
                i0 = toff[g] * 128
                XL = edgex.tile([128, TPDmax * SLE], GDT, tag="XL", name="XL")
                XL3 = XL[:, :tpd * SLE].rearrange("p (t c) -> p t c", c=SLE)
                gather_rows(XL3, xl_full, srcidx, i0, tpd * 128, SLE)

                ht = edge.tile([128, TPDmax * HID], GDT, tag="ht", name="ht")
                ht3 = ht[:, :tpd * HID].rearrange("p (t c) -> p t c", c=HID)
                xr_b = xr_sb[:, g * HID:(g + 1) * HID].unsqueeze(1) \
                    .broadcast_to([128, tpd, HID])
                nc.vector.tensor_tensor(ht3, XL3[:, :, 0:HID], xr_b, ALU.add)

                # sumA|h| - sumB|h| = (R0 - R1) + s*2*Rsmall, via ONE contiguous
                # interleaved abs-reduce over [128, 2*tpd, 128] (full 2x rate)
                # plus a tiny correction slice [min(nA,128), max(nA,128)).
                red = col.tile([128, 2 * TPDmax], F32, tag="red", name="red")
                ht2 = ht[:, :tpd * HID].rearrange("p (t c) -> p t c", c=128)
                nc.vector.tensor_reduce(red[:, :2 * tpd], ht2, AX.X, ALU.add,
                                        apply_absolute_value=True)
                eS = col.tile([128, TPDmax], F32, tag="eS", name="eS")
                lo, hi = min(nA, 128), max(nA, 128)
                sgn = -2.0 if nA < 128 else 2.0
                nc.vector.tensor_reduce(eS[:, :tpd], ht3[:, :, lo:hi], AX.X, ALU.add,
                                        apply_absolute_value=True)
                SLr = col.tile([128, TPDmax], F32, tag="SLr", name="SLr")
                nc.vector.tensor_reduce(SLr[:, :tpd], XL3, AX.X, ALU.add)
                red3 = red[:, :2 * tpd].rearrange("p (t two) -> p t two", two=2)
                d1 = col.tile([128, TPDmax], F32, tag="d1", name="d1")
                d1v = d1[:, :tpd].rearrange("p (t o) -> p t o", o=1)
                nc.vector.tensor_tensor(d1v, red3[:, :, 0:1], red3[:, :, 1:2],
                                        ALU.subtract)
                d2 = col.tile([128, TPDmax], F32, tag="d2", name="d2")
                nc.vector.scalar_tensor_tensor(d2[:, :tpd], eS[:, :tpd], sgn,
                                               d1[:, :tpd], ALU.mult, ALU.add)
                earg = col.tile([128, TPDmax], F32, tag="earg", name="earg")
                nc.vector.scalar_tensor_tensor(earg[:, :tpd], SLr[:, :tpd], 1.5,
                                               d2[:, :tpd], ALU.mult, ALU.add)
                wexp = col.tile([128, TPDmax], GDT, tag="wexp", name="wexp")
                nc.scalar.activation(wexp[:, :tpd], earg[:, :tpd], AF.Exp, scale=0.4)

                diagw = edge.tile([128, TPDmax * 128], GDT, tag="diagw", name="diagw")
                dw3 = diagw[:, :tpd * 128].rearrange("p (t c) -> p t c", c=128)
                nc.vector.tensor_tensor(
                    dw3,
                    ident_bf[:].unsqueeze(1).broadcast_to([128, tpd, 128]),
                    wexp[:, :tpd].unsqueeze(2).broadcast_to([128, tpd, 128]),
                    ALU.mult)

                ps_g = psum.tile([128, HID], F32, tag="nmm", name="agg")
                ps1 = psum.tile([128, 1], F32, tag="e1", name="agg1")
                for j in range(tpd):
                    nc.tensor.matmul(ps_g[:], lhsT=diagw[:, j * 128:(j + 1) * 128],
                                     rhs=XL[:, j * SLE:j * SLE + HID],
                                     start=(j == 0), stop=(j == tpd - 1))
                    nc.tensor.matmul(ps1[:], lhsT=diagw[:, j * 128:(j + 1) * 128],
                                     rhs=ones_bfc[:],
                                     start=(j == 0), stop=(j == tpd - 1))

                dce = col.tile([128, 1], F32, tag="dce", name="dce")
                nc.vector.tensor_scalar(dce[:], ps1[:], 1e-16, None, ALU.add)
                rcol = col.tile([128, 1], F32, tag="rcol", name="rcol")
                nc.vector.reciprocal(rcol[:], dce[:])
                msg = work.tile([128, HID], GDT, tag="msg", name="msg")
                nc.vector.tensor_scalar(msg[:], ps_g[:], rcol[:], None, ALU.mult)
                for mc in range(2):
                    ps = psum.tile([128, 128], GDT, tag="tr", name="tr")
                    nc.tensor.matmul(ps[:], lhsT=msg[:, mc * 128:(mc + 1) * 128],
                                     rhs=ident_bf[:], is_transpose=True,
                                     start=True, stop=True)
                    evac_block(raw[mc], ps[:], g * 128, (g + 1) * 128, mc,
                               st1, st2, g, scalar1=rattp[mc][:, li:li + 1],
                               scalar2=zero_col[:], op0=ALU.mult, op1=ALU.add)
            rs_l, nmr_l = stats_and_norm(st1, st2, ar_in[2 + li], ar_out[2 + li],
                                         cfg.N, NDT)
            norm_gelu(raw, xt, rs_l, nmr_l)
        edgex.release()
        edge.release()

        # =================== pooling ===================
        pt = work.tile([128, 4], F32, tag="pt", name="pt")
        for m in range(2):
            nc.vector.tensor_reduce(pt[:, m:m + 1], xt[m][:, :NS], AX.X, ALU.add)
            nc.vector.tensor_reduce(pt[:, 2 + m:3 + m], xt[m][:, :NS], AX.X, ALU.max)
        nc.sync.dma_start(out=pool_in[:], in_=pt[:])
        nc.gpsimd.collective_compute(
            "AllGather", ALU.bypass, replica_groups=REPL,
            ins=[pool_in[:]], outs=[pool_out[:]])
        pg = work.tile([128, NCORES * 4], F32, tag="pg", name="pg")
        nc.sync.dma_start(out=pg[:].rearrange("p (k v) -> p k v", v=4),
                          in_=pool_out[:].rearrange("(k p) v -> p k v", p=128))
        pg3 = pg[:].rearrange("p (k v) -> p k v", v=4)
        pool_c = []
        for m in range(2):
            s_ = col.tile([128, 1], F32, tag="psum_c", name="psum_c")
            nc.vector.tensor_reduce(s_[:], pg3[:, :, m], AX.X, ALU.add)
            mx = col.tile([128, 1], F32, tag="pmax_c", name="pmax_c")
            nc.vector.tensor_reduce(mx[:], pg3[:, :, 2 + m], AX.X, ALU.max)
            pc = col.tile([128, 1], F32, tag="pool_c", name="pool_c")
            nc.vector.scalar_tensor_tensor(pc[:], s_[:], 1.0 / cfg.N, mx[:],
                                           ALU.mult, ALU.add)
            pool_c.append(pc)

        # =================== late MLP (replicated) ===================
        cblocks = [(s, min(s + 512, cfg.CP)) for s in range(0, cfg.CP, 512)]
        late = tc.alloc_tile_pool(name="late", bufs=1)
        cfT = late.tile([CF, cfg.CP], F32, tag="cfT", name="cfT")
        nc.sync.dma_start(out=cfT[:], in_=cft_d[:])

        vcol = []
        for mc in range(2):
            ps = psum.tile([128, 1], F32, tag="e1", name="e1")
            for kc in range(2):
                nc.tensor.matmul(ps[:], lhsT=w1p[kc][:, mc * 128:(mc + 1) * 128],
                                 rhs=pool_c[kc][:], start=(kc == 0), stop=(kc == 1))
            v = col.tile([128, 1], F32, tag="vcol", name="vcol")
            nc.vector.tensor_tensor(v[:], ps[:], bc0c[mc][:], ALU.add)
            vcol.append(v)

        h1 = [late.tile([128, cfg.CP], F32, tag=f"h1_{m}", name=f"h1_{m}") for m in range(2)]
        h2 = [late.tile([128, cfg.CP], F32, tag=f"h2_{m}", name=f"h2_{m}") for m in range(2)]

        def cfg_stats_norm(tiles, st1, st2, nblk, two_chunks):
            rs_l, nmr_l = [], []
            for m in range(2 if two_chunks else 1):
                s1 = col.tile([128, 1], F32, tag="cs1", name="cs1")
                s2c = col.tile([128, 1], F32, tag="cs2", name="cs2")
                nc.vector.tensor_reduce(s1[:], st1[m][:, :nblk], AX.X, ALU.add)
                nc.vector.tensor_reduce(s2c[:], st2[m][:, :nblk], AX.X, ALU.add)
                mu = col.tile([128, 1], F32, tag="mu", name="mu")
                nc.vector.tensor_scalar(mu[:], s1[:], 1.0 / cfg.C, None, ALU.mult)
                mu2 = col.tile([128, 1], F32, tag="mu2", name="mu2")
                nc.scalar.activation(mu2[:], mu[:], AF.Square)
                var = col.tile([128, 1], F32, tag="var", name="var")
                nc.vector.scalar_tensor_tensor(var[:], s2c[:], 1.0 / cfg.C, mu2[:],
                                               ALU.mult, ALU.subtract)
                sd = col.tile([128, 1], F32, tag="sd", name="sd")
                nc.scalar.activation(sd[:], var[:], AF.Sqrt, bias=eps_col[:])
                rs = col.tile([128, 1], F32, tag="rs", name="rs")
                nc.vector.reciprocal(rs[:], sd[:])
                nmr = col.tile([128, 1], F32, tag="nmr", name="nmr")
                nc.vector.tensor_scalar(nmr[:], mu[:], rs[:], -1.0, ALU.mult, ALU.mult)
                rs_l.append(rs)
                nmr_l.append(nmr)
            for m in range(2 if two_chunks else 1):
                nc.scalar.activation(tiles[m][:], tiles[m][:], AF.Gelu,
                                     bias=nmr_l[m][:], scale=rs_l[m][:])

        st1 = stats_tiles("l1s1")
        st2 = stats_tiles("l1s2")
        for mc in range(2):
            vc = vcol[mc]
            for bi, (s, e) in enumerate(cblocks):
                w = e - s
                ps = psum.tile([128, 512], F32, tag="mm", name="mm")
                nc.tensor.matmul(ps[:, :w], lhsT=w1c[:, mc * 128:(mc + 1) * 128],
                                 rhs=cfT[:, s:e], start=True, stop=True)

                def cone(a, b, accum):
                    kw = {'accum_out': st1[mc][:, bi:bi + 1]} if accum else {}
                    nc.vector.tensor_scalar(h1[mc][:, a:b], ps[:, a - s:b - s],
                                            vc[:], 0.0, ALU.add, ALU.add, **kw)
                    if accum:
                        sq = work.tile([128, 512], F32, tag="sqscr", name="sqscr")
                        nc.scalar.activation(sq[:, :b - a], h1[mc][:, a:b], AF.Square,
                                             accum_out=st2[mc][:, bi:bi + 1])
                if s >= cfg.C:
                    cone(s, e, False)
                elif e <= cfg.C:
                    cone(s, e, True)
                else:
                    cone(s, cfg.C, True)
                    cone(cfg.C, e, False)
        cfg_stats_norm(h1, st1, st2, len(cblocks), True)

        st1 = stats_tiles("l2s1")
        st2 = stats_tiles("l2s2")
        for bi, (s, e) in enumerate(cblocks):
            w = e - s
            ps = psum.tile([128, 512], F32, tag="mm", name="mm")
            for kc in range(2):
                nc.tensor.matmul(ps[:, :w], lhsT=w2l[kc][:], rhs=h1[kc][:, s:e],
                                 start=(kc == 0), stop=(kc == 1))

            def done(a, b, accum):
                kw = {'accum_out': st1[0][:, bi:bi + 1]} if accum else {}
                nc.vector.tensor_scalar(h2[0][:, a:b], ps[:, a - s:b - s],
                                        0.0, 0.0, ALU.add, ALU.add, **kw)
                if accum:
                    sq = work.tile([128, 512], F32, tag="sqscr", name="sqscr")
                    nc.scalar.activation(sq[:, :b - a], h2[0][:, a:b], AF.Square,
                                         accum_out=st2[0][:, bi:bi + 1])
            if s >= cfg.C:
                done(s, e, False)
            elif e <= cfg.C:
                done(s, e, True)
            else:
                done(s, cfg.C, True)
                done(cfg.C, e, False)
        cfg_stats_norm(h2, st1, st2, len(cblocks), False)

        outsb = late.tile([1, cfg.CP], F32, tag="outsb", name="outsb")
        for (s, e) in cblocks:
            w = e - s
            ps = psum.tile([1, 512], F32, tag="mm", name="predps")
            nc.tensor.matmul(ps[:, :w], lhsT=predw[:], rhs=h2[0][:, s:e],
                             start=True, stop=True)
            nc.vector.tensor_scalar(outsb[:, s:e], ps[:, :w], predb[:],
                                    None, ALU.add)
        nc.sync.dma_start(out=out_d[:], in_=outsb[:])
        late.release()

    nc.compile()
    return nc


# ---------------------------------------------------------------------------
# entry point
# ---------------------------------------------------------------------------

_prog_cache = {}


def kernel(**inputs) -> np.ndarray:
    cfg, in_maps = host_prep(inputs)
    key = cfg.key()
    if key not in _prog_cache:
        _prog_cache[key] = build_program(cfg)
    nc = _prog_cache[key]
    res = run_bass_kernel_spmd(nc, in_maps, list(range(NCORES)))
    out = np.asarray(res.results[0]['out']).reshape(-1)[:cfg.C]
    return out.astype(np.float32)
